# revision 7
# baseline (speedup 1.0000x reference)
"""Trainium2 Bass kernel for a 4-layer alternating-direction LSTM encoder + FFN.

Problem shapes (hardcoded): B=64, T=512, H=512, F=2048, L=4, gates 4H=2048.

Strategy: data-parallel over batch across 8 cores (8 examples/core). All
on-device tensors live in a transposed layout (feature dim on the 128 SBUF
partitions, (time, batch) on the free dim) so the per-timestep elementwise
work runs at full 128-lane width. Per layer, three phases:

  A) XwT[t] = (Wx^T x_t + b)  for all t   -- one big batched matmul -> DRAM
  B) sequential LSTM recurrence over T steps. Per step, each gate gets its
     own PSUM bank: first an identity-stationary matmul injects the
     precomputed XwT slice (start=True), then 16 Wh-stationary matmuls
     accumulate Wh^T h_{t-1} on top. Gate order i,g,f,o lets the
     sigmoid/tanh + cell-update chain run concurrently with the remaining
     gate matmuls; the post-matmul tail is just sigmoid(o) and h=o*tanh(c).
  C) FFN: a2 = W2^T relu(W1^T h + b1) + b2  -- batched matmuls.

Layer direction flips are handled purely by index order (read-side reversal
of XwT blocks in phase B + reversed staging-slot order), data always stays in
global time order. Host side only reshapes/casts (sharding + layout prep).
"""

import numpy as np
import ml_dtypes
from contextlib import ExitStack

import concourse.bass as bass
from concourse import bacc
import concourse.mybir as mybir
import concourse.tile as tile
from concourse.bass import ds, ts
from concourse.bass_utils import run_bass_kernel_spmd

BF16 = mybir.dt.bfloat16
F32 = mybir.dt.float32
AF = mybir.ActivationFunctionType
ALU = mybir.AluOpType

B, T, H, F, L = 64, 512, 512, 2048, 4
NCORES = 8
BL = B // NCORES          # 8 examples per core
NT = BL * T               # 4096 free-dim columns (t-major: col = t*BL + b)
KH = H // 128             # 4 contraction chunks over H
MG = (4 * H) // 128       # 16 gate-dim m-tiles
KF = F // 128             # 16 contraction chunks over F
TBLK = 32                 # recurrence steps per staged block
NBLK = T // TBLK          # 16 blocks
BODY = 2 * TBLK           # 64 steps per body (2 blocks)
CHUNK = 512               # batched-matmul moving free dim
NCHUNK = NT // CHUNK      # 8
XSLOT = MG * TBLK * BL    # 4096 xwt cols per time-block
XPAD = 2 * XSLOT          # over-alloc pad on both ends (prefetch overrun)

_built = None
DEBUG = False
TRACE = False
last_results = None
import os as _os
REPEAT = int(_os.environ.get("BASS_LSTM_REPEAT", "1"))
UNROLL = _os.environ.get("BASS_LSTM_UNROLL") == "1"
SKIP_B = _os.environ.get("BASS_LSTM_SKIP_B") == "1"

# processing order (o last so the tail is just sigmoid(o), h=o*tanh(c));
# mbase = first m-tile of the gate in the i,f,g,o-ordered 4H dim.
# i and f share one [128, 1024] psum tile spanning two banks (i at cols
# 0:32, f at 512:544) so one Activation computes both sigmoids.
GATE_ORDER = (("i", 0), ("f", 4), ("g", 8), ("o", 12))


def _build_nc():
    nc = bacc.Bacc(None, target_bir_lowering=False)

    xT = nc.declare_dram_parameter("xT", [KH, 128, NT], BF16, isOutput=False)
    wx = nc.declare_dram_parameter("wx", [L, 128, KH, 4 * H], BF16, isOutput=False)
    wh = nc.declare_dram_parameter("wh", [L, 128, KH, 4 * H], BF16, isOutput=False)
    gb = nc.declare_dram_parameter("gb", [L, 128, MG], F32, isOutput=False)
    w1 = nc.declare_dram_parameter("w1", [L, 128, KH, F], BF16, isOutput=False)
    b1v = nc.declare_dram_parameter("b1v", [L, 128, KF], F32, isOutput=False)
    w2 = nc.declare_dram_parameter("w2", [L, 128, KF, H], BF16, isOutput=False)
    b2v = nc.declare_dram_parameter("b2v", [L, 128, KH], F32, isOutput=False)
    identv = nc.declare_dram_parameter("identv", [128, 128], BF16, isOutput=False)
    out = nc.declare_dram_parameter("out", [KH, 128, NT], F32, isOutput=True)

    # XwT scratch, flattened 2D so all dynamic-offset DMAs stay contiguous:
    # slot s (time-block) at cols [XPAD + s*XSLOT, ...), m-major inside.
    xwt = nc.dram_tensor("xwt", [128, 2 * XPAD + NBLK * XSLOT], BF16)

    with tile.TileContext(nc) as tc, ExitStack() as ctx:
        wpool = ctx.enter_context(tc.tile_pool(name="weights", bufs=1))
        state = ctx.enter_context(tc.tile_pool(name="state", bufs=1))
        small = ctx.enter_context(tc.tile_pool(name="small", bufs=2))
        xwp = ctx.enter_context(tc.tile_pool(name="xwp", bufs=1))
        tmpa = ctx.enter_context(tc.tile_pool(name="tmpa", bufs=3))
        a1p = ctx.enter_context(tc.tile_pool(name="a1p", bufs=1))
        ewp = ctx.enter_context(tc.tile_pool(name="ewp", bufs=2))
        pscr = ctx.enter_context(tc.tile_pool(name="pscr", bufs=3, space="PSUM"))
        gp = ctx.enter_context(tc.tile_pool(name="gp", bufs=1, space="PSUM"))

        # Persistent state: two ping-pong sequence buffers (feature-transposed,
        # [128, KH, NT]), the recurrence staging buffer and cell state.
        bufs = [state.tile([128, KH, NT], BF16, tag=f"seq{i}", name=f"seq{i}") for i in range(2)]
        # staging: [carry 32][64 slots x 32]; slot q at cols 32+q*32,
        # col layout within a slot: (hchunk, b)
        stag = state.tile([128, 32 + BODY * 32], BF16, tag="stag")
        cst = [state.tile([128, 32], F32, tag=f"c{i}", name=f"c{i}") for i in range(2)]
        ident_sb = state.tile([128, 128], BF16, tag="ident")
        nc.sync.dma_start(out=ident_sb, in_=identv[:, :])
        # xw double buffer (explicit ping-pong across bodies)
        xwa = xwp.tile([128, 2 * XSLOT], BF16, tag="xwa", name="xwa")
        xwb = xwp.tile([128, 2 * XSLOT], BF16, tag="xwb", name="xwb")

        import contextlib
        rep_ctx = tc.For_i(0, REPEAT, 1) if REPEAT > 1 else contextlib.nullcontext()
        with rep_ctx:
            _build_pass(nc, tc, locals())

    nc.finalize()
    return nc


def _build_pass(nc, tc, env):
    xT, wx, wh, gb, w1, b1v, w2, b2v, out = (
        env["xT"], env["wx"], env["wh"], env["gb"], env["w1"],
        env["b1v"], env["w2"], env["b2v"], env["out"])
    xwt = env["xwt"]
    wpool, state, small, xwp, tmpa, a1p, ewp, pscr, gp = (
        env["wpool"], env["state"], env["small"], env["xwp"],
        env["tmpa"], env["a1p"], env["ewp"], env["pscr"], env["gp"])
    bufs, stag, cst, ident_sb = env["bufs"], env["stag"], env["cst"], env["ident_sb"]
    xwa, xwb = env["xwa"], env["xwb"]

    for layer in range(L):
        rev = layer % 2 == 1
        buf_in = bufs[0]   # layer input; FFN writes its output back here
        buf_out = bufs[1]  # recurrence hidden states (global time order)

        # ---- weights + biases for this layer ----
        wx_sb = wpool.tile([128, KH, 4 * H], BF16, tag="wx")
        wh_sb = wpool.tile([128, KH, 4 * H], BF16, tag="wh")
        w1_sb = wpool.tile([128, KH, F], BF16, tag="w1")
        w2_sb = wpool.tile([128, KF, H], BF16, tag="w2")
        nc.sync.dma_start(out=wx_sb, in_=wx[layer])
        nc.sync.dma_start(out=wh_sb, in_=wh[layer])
        nc.sync.dma_start(out=w1_sb, in_=w1[layer])
        nc.sync.dma_start(out=w2_sb, in_=w2[layer])
        gb_sb = small.tile([128, MG], F32, tag="gb")
        b1_sb = small.tile([128, KF], F32, tag="b1")
        b2_sb = small.tile([128, KH], F32, tag="b2")
        nc.sync.dma_start(out=gb_sb, in_=gb[layer])
        nc.sync.dma_start(out=b1_sb, in_=b1v[layer])
        nc.sync.dma_start(out=b2_sb, in_=b2v[layer])

        if layer == 0:
            for k in range(KH):
                nc.sync.dma_start(out=buf_in[:, k, :], in_=xT[k])

        # ---------------- Phase A: XwT = Wx^T @ in + b ----------------
        for c in range(NCHUNK):
            cols = ds(c * CHUNK, CHUNK)
            for m in range(MG):
                pt = pscr.tile([128, CHUNK], F32, tag="ps")
                for k in range(KH):
                    nc.tensor.matmul(
                        pt,
                        wx_sb[:, k, ts(m, 128)],
                        buf_in[:, k, cols],
                        start=(k == 0),
                        stop=(k == KH - 1),
                    )
                sb = tmpa.tile([128, CHUNK], BF16, tag="xa")
                nc.scalar.activation(sb, pt, AF.Identity, bias=gb_sb[:, m : m + 1])
                # chunk c covers time-blocks 2c, 2c+1
                o0 = XPAD + (2 * c) * XSLOT + m * 256
                o1 = XPAD + (2 * c + 1) * XSLOT + m * 256
                nc.sync.dma_start(out=xwt[:, ds(o0, 256)], in_=sb[:, 0:256])
                nc.sync.dma_start(out=xwt[:, ds(o1, 256)], in_=sb[:, 256:512])

        # ---------------- Phase B: recurrence ----------------
        if SKIP_B:
            continue
        nc.vector.memset(stag[:, 0:32], 0.0)
        nc.vector.memset(cst[0], 0.0)

        def slot_w(s):
            # staging slot written by step s (global-time order inside body)
            k, p = divmod(s, TBLK)
            return k * TBLK + (TBLK - 1 - p) if rev else s

        def stag_cols(q):
            return ds(32 + q * 32, 32)

        def xw_base(jvv):
            # xwt col base of the body covering processing-order blocks
            # (jvv, jvv+1); jvv may be a register expression.
            if rev:
                return XPAD + (NBLK - 2) * XSLOT - jvv * XSLOT
            return XPAD + jvv * XSLOT

        def run_body(jvv, xwbuf):
            """64 recurrence steps reading the XwT body staged in xwbuf."""
            xwv = xwbuf.rearrange("p (h m c) -> p h m c", h=2, m=MG)
            for s in range(BODY):
                blk = s // TBLK
                pos = s % TBLK
                scol = (TBLK - 1 - pos) if rev else pos
                half = (1 - blk) if rev else blk
                rd_base = (32 + slot_w(s - 1) * 32) if s > 0 else 0

                # per-gate PSUM banks; inject XwT via identity-stationary
                # matmuls first (they don't depend on h, so they run during
                # the previous step's tail), then accumulate Wh^T h.
                pif = gp.tile([128, 1024], F32, tag="pif", name="pif")
                pg_t = gp.tile([128, 32], F32, tag="pg", name="pg_t")
                po_t = gp.tile([128, 32], F32, tag="po", name="po_t", bufs=2)
                gviews = {"i": pif[:, 0:32], "f": pif[:, 512:544],
                          "g": pg_t, "o": po_t}
                for g, mb in GATE_ORDER:
                    nc.tensor.matmul(
                        gviews[g],
                        ident_sb,
                        xwv[:, half, mb : mb + 4, ds(scol * 8, 8)],
                        start=True,
                        stop=False,
                        skip_group_check=True,
                    )
                for g, mb in GATE_ORDER:
                    pt = gviews[g]
                    for hc in range(4):
                        for k in range(KH):
                            nc.tensor.matmul(
                                pt[:, ds(hc * 8, 8)],
                                wh_sb[:, k, ts(mb + hc, 128)],
                                stag[:, ds(rd_base + k * 8, 8)],
                                start=False,
                                stop=(hc == 3 and k == KH - 1),
                                skip_group_check=True,
                            )

                c_cur, c_nxt = cst[s % 2], cst[(s + 1) % 2]
                a_if = ewp.tile([128, 64], F32, tag="aif", name="a_if")
                a_g = ewp.tile([128, 32], F32, tag="ag", name="a_g")
                a_o = ewp.tile([128, 32], F32, tag="ao", name="a_o")
                thc = ewp.tile([128, 32], F32, tag="thc", name="thc")
                igt = ewp.tile([128, 32], F32, tag="ig", name="igt")
                fct = ewp.tile([128, 32], F32, tag="fc", name="fct")
                # ACT queue: sig(i,f) in one op, tanh(g), tanh(c), sig(o);
                # tanh(c) runs while the o-gate matmuls are still streaming
                nc.scalar.activation(
                    a_if,
                    pif.rearrange("p (b x) -> p b x", x=512)[:, :, 0:32],
                    AF.Sigmoid,
                )
                nc.scalar.activation(a_g, pg_t, AF.Tanh)
                # DVE queue
                nc.vector.tensor_tensor(igt, a_if[:, 0:32], a_g, ALU.mult)
                nc.vector.tensor_tensor(fct, a_if[:, 32:64], c_cur, ALU.mult)
                nc.vector.tensor_tensor(c_nxt, fct, igt, ALU.add)
                nc.scalar.activation(thc, c_nxt, AF.Tanh)
                nc.scalar.activation(a_o, po_t, AF.Sigmoid)
                nc.vector.tensor_tensor(
                    stag[:, stag_cols(slot_w(s))], a_o, thc, ALU.mult
                )

            # carry h_{last} into cols [0:32] for the next body (first!)
            nc.vector.tensor_copy(stag[:, 0:32], stag[:, stag_cols(slot_w(BODY - 1))])
            # flush the two staged blocks into buf_out (global time order)
            sv = stag[:, 32 : 32 + BODY * 32].rearrange("p (q x) -> p q x", x=32)
            for sblk in range(2):
                if rev:
                    gbase = (NBLK - 1) * 256 - (jvv + sblk) * 256
                else:
                    gbase = (jvv + sblk) * 256
                for hc in range(KH):
                    dst = buf_out[:, hc, ds(gbase, 256)].rearrange(
                        "p (q x) -> p q x", x=8
                    )
                    nc.vector.tensor_copy(dst, sv[:, ts(sblk, TBLK), ds(hc * 8, 8)])

        # preload first body, then run bodies two at a time with the next
        # body's XwT prefetched into the other buffer.
        nc.sync.dma_start(out=xwa, in_=xwt[:, ds(xw_base(0), 2 * XSLOT)])

        def loop_body(jv):
            nc.sync.dma_start(out=xwb, in_=xwt[:, ds(xw_base(jv + 2), 2 * XSLOT)])
            run_body(jv, xwa)
            nc.sync.dma_start(out=xwa, in_=xwt[:, ds(xw_base(jv + 4), 2 * XSLOT)])
            run_body(jv + 2, xwb)

        if UNROLL:
            for jv in range(0, NBLK, 4):
                loop_body(jv)
        else:
            with tc.For_i(0, NBLK, 4, hint_engines=(mybir.EngineType.PE,)) as jv:
                loop_body(jv)

        # ---------------- Phase C: FFN ----------------
        last = layer == L - 1
        for c in range(NCHUNK):
            cols = ds(c * CHUNK, CHUNK)
            a1 = a1p.tile([128, KF, CHUNK], BF16, tag="a1")
            for m in range(KF):
                pt = pscr.tile([128, CHUNK], F32, tag="ps")
                for k in range(KH):
                    nc.tensor.matmul(
                        pt,
                        w1_sb[:, k, ts(m, 128)],
                        buf_out[:, k, cols],
                        start=(k == 0),
                        stop=(k == KH - 1),
                    )
                nc.scalar.activation(
                    a1[:, m, :], pt, AF.Relu, bias=b1_sb[:, m : m + 1]
                )
            for mo in range(KH):
                pt = pscr.tile([128, CHUNK], F32, tag="ps")
                for k in range(KF):
                    nc.tensor.matmul(
                        pt,
                        w2_sb[:, k, ts(mo, 128)],
                        a1[:, k, :],
                        start=(k == 0),
                        stop=(k == KF - 1),
                    )
                if last:
                    ot = tmpa.tile([128, CHUNK], F32, tag="oc")
                    nc.scalar.activation(ot, pt, AF.Identity, bias=b2_sb[:, mo : mo + 1])
                    nc.sync.dma_start(out=out[mo, :, cols], in_=ot)
                else:
                    nc.scalar.activation(
                        buf_in[:, mo, cols], pt, AF.Identity,
                        bias=b2_sb[:, mo : mo + 1],
                    )


def _get_nc():
    global _built
    if _built is None:
        _built = _build_nc()
    return _built


def kernel(**inputs):
    x = np.asarray(inputs["x"], np.float32)
    Wx = np.asarray(inputs["Wx"], np.float32)
    Wh = np.asarray(inputs["Wh"], np.float32)
    b = np.asarray(inputs["b"], np.float32)
    W1 = np.asarray(inputs["W1"], np.float32)
    b1 = np.asarray(inputs["b1"], np.float32)
    W2 = np.asarray(inputs["W2"], np.float32)
    b2 = np.asarray(inputs["b2"], np.float32)

    bf = ml_dtypes.bfloat16
    wx_h = np.ascontiguousarray(Wx.reshape(L, KH, 128, 4 * H).transpose(0, 2, 1, 3)).astype(bf)
    wh_h = np.ascontiguousarray(Wh.reshape(L, KH, 128, 4 * H).transpose(0, 2, 1, 3)).astype(bf)
    gb_h = np.ascontiguousarray(b.reshape(L, MG, 128).transpose(0, 2, 1)).astype(np.float32)
    w1_h = np.ascontiguousarray(W1.reshape(L, KH, 128, F).transpose(0, 2, 1, 3)).astype(bf)
    b1_h = np.ascontiguousarray(b1.reshape(L, KF, 128).transpose(0, 2, 1)).astype(np.float32)
    w2_h = np.ascontiguousarray(W2.reshape(L, KF, 128, H).transpose(0, 2, 1, 3)).astype(bf)
    b2_h = np.ascontiguousarray(b2.reshape(L, KH, 128).transpose(0, 2, 1)).astype(np.float32)
    ident_h = np.eye(128, dtype=bf)

    in_maps = []
    for c in range(NCORES):
        xc = x[c * BL : (c + 1) * BL]  # [BL, T, H]
        # xT[hc, p, t*BL + b] = xc[b, t, hc*128+p]
        xt = np.ascontiguousarray(
            xc.reshape(BL, T, KH, 128).transpose(2, 3, 1, 0).reshape(KH, 128, NT)
        ).astype(bf)
        in_maps.append(
            dict(
                xT=xt, wx=wx_h, wh=wh_h, gb=gb_h,
                w1=w1_h, b1v=b1_h, w2=w2_h, b2v=b2_h, identv=ident_h,
            )
        )

    nc = _get_nc()
    global last_results
    import kernel as _K
    _K.kernel_prepped_maps = in_maps
    kr = run_bass_kernel_spmd(
        nc, in_maps, core_ids=list(range(NCORES)), trace=TRACE
    )
    last_results = kr
    res = kr.results

    outp = np.empty((B, T, H), np.float32)
    for c in range(NCORES):
        oc = res[c]["out"]  # [KH, 128, NT] f32
        outp[c * BL : (c + 1) * BL] = (
            oc.reshape(KH, 128, T, BL).transpose(3, 2, 0, 1).reshape(BL, T, H)
        )
    return outp


if __name__ == "__main__":
    _get_nc()
    print("build ok")
